# revision 7
# baseline (speedup 1.0000x reference)
"""BPCA pooling kernel for Trainium2 (Bass/Tile), 8-core data-parallel.

Per core: 4 images [128,128,64] f32.
  1. SWDGE cast-DMA each image HBM->SBUF fp32->fp16 into layout
     [128 part = (hh, jh), free = (dy, jl, dx, c4, a)] so that 4-channel
     classes (a = c%4) are innermost and HBM reads are 16KB-contiguous.
  2. Per-image 4x4 Gram + class sums on PE in fp16 (fp32 PSUM): 64 chunk
     self-matmuls [128,128] accumulate the 32 diagonal 4x4 blocks; an
     interleaved N=1 ones-matmul accumulates per-column sums.
  3. Fold on PE: mask off-diagonal blocks (DVE), em4^T-matmul collapses
     32 diag blocks + the sums column; small DVE reduce; tiny SBUF->SBUF
     gather DMA into image-major [2, 20] pair tiles.
  4. Eigen chain per image-pair on DVE: standardize Gram -> Ghat/4,
     center + Frobenius-shift, 10 power-squarings (B <- B@B via
     broadcast-mult + strided reduce), column-sum = sign-fixed top
     eigenvector, Newton-normalized; w = v/sigma, b = -w.m.
  5. Projection: out = sum_a w_a x_a + b. Front 1280 groups on DVE in
     fp16 (bcast-mult + group-4 reduce + bias), back 768 groups on ACT
     (strided scaled-identity per class) + GPSIMD/DVE adds.
  6. Output DMA [128, 2048] f32 with 8KB-contiguous HBM writes.
"""

import sys
from contextlib import ExitStack

import numpy as np

for _p in ("/opt/trn_rl_repo",):
    if _p not in sys.path:
        sys.path.insert(0, _p)

import concourse.bass as bass
import concourse.bacc as bacc
import concourse.tile as tile
from concourse import mybir
from concourse.bass_utils import run_bass_kernel_spmd

AF = mybir.ActivationFunctionType
OP = mybir.AluOpType
AX = mybir.AxisListType
F32 = mybir.dt.float32
F16 = mybir.dt.float16

B, H, W, C = 32, 128, 128, 64
NCORES = 8
IMGS = B // NCORES  # 4 images per core
NROWS = float(H * W * C // 4)  # 262144 rows per image
SQRTN = float(np.sqrt(NROWS))
FREE = H * W * C // 128  # 8192 elems per partition per image
NCHUNK = FREE // 128  # 64 gram chunks per image
NSQ = 10  # power-squarings on the centered+shifted Ghat
G1 = 1280  # groups on the DVE projection path (of 2048)
G2 = 2048 - G1  # groups on the ACT+GPS projection path


def _load_image(nc, x, X5, i):
    # HBM: x[i] [h=128, w=128, c=64] fp32, h=(hh,dy), w=(jh,jl,dx), c=(c4,a)
    # SBUF: X5 [128=(jh,hh), 8192=(jl,dy,dx,c4,a)] fp16
    # 4 cast-DMAs (dy x jh), each 16KB-contiguous HBM reads per partition.
    for dy in range(2):
        for jh in range(2):
            src = x[i][dy::2, jh * 64 : (jh + 1) * 64, :].rearrange(
                "hh (jl dx) c -> hh jl (dx c)", dx=2
            )  # [64, 32, 128]
            dst = (
                X5[jh * 64 : (jh + 1) * 64]
                .rearrange("p (jl dy r) -> p jl dy r", jl=32, dy=2)[:, :, dy, :]
            )  # [64, 32, 128]
            nc.gpsimd.dma_start(dst, src)


def _gram(nc, gpool, X5, ones1, i):
    gp = gpool.tile([128, 132], F32, name=f"gram{i}", tag="gram")
    for k in range(NCHUNK):
        chunk = X5[:, k * 128 : (k + 1) * 128]
        nc.tensor.matmul(gp[:, 0:128], chunk, chunk,
                         start=(k == 0), stop=False)
        nc.tensor.matmul(gp[:, 128:129], chunk, ones1[:],
                         start=False, stop=(k == NCHUNK - 1))
    return gp


def _fold(ctx, tc, pools, gp, mask, em4, Gpair, q, i):
    nc = tc.nc
    v = nc.vector
    act = nc.scalar
    spool, p2pool = pools
    # PSUM -> SBUF copy on ACT; mask diag blocks on DVE
    gs = spool.tile([128, 129], F32, name=f"gs{i}", tag="gs")
    act.activation(gs[:], gp[:, 0:129], AF.Identity)
    gm = spool.tile([128, 128], F32, name=f"gm{i}", tag="gm")
    v.tensor_tensor(gm[:], gs[:, 0:128], mask[:], OP.mult)
    # fold 32 diag blocks + sums column via em4^T matmul
    ps2 = p2pool.tile([4, 132], F32, name=f"ps2_{i}", tag="ps2")
    nc.tensor.matmul(ps2[:, 0:128], em4[:], gm[:], start=True, stop=False)
    nc.tensor.matmul(ps2[:, 128:129], em4[:], gs[:, 128:129],
                     start=False, stop=True)
    f5 = spool.tile([4, 129], F32, name=f"f5_{i}", tag="f5")
    act.activation(f5[:], ps2[:, 0:129], AF.Identity)
    G4s = spool.tile([4, 4], F32, name=f"g4s{i}", tag="g4s")
    gv = f5[:, 0:128].rearrange("p (j b) -> p j b", b=4).transpose([0, 2, 1])
    v.tensor_reduce(G4s[:], gv, AX.X, OP.add)
    # gather into image-major pair tile: [2, 20] = (G[a,b] at 4a+b, s_a at 16+a)
    nc.sync.dma_start(Gpair[q : q + 1, 0:16], G4s[:])
    nc.sync.dma_start(Gpair[q : q + 1, 16:20], f5[:, 128:129])


def _chain(tc, jp, Gpair, p, dbg=None):
    """Pair eigen chain on [2,*] tiles -> wbc [128, 10] f32 (w0..w3, b)."""
    nc = tc.nc
    v = nc.vector
    act = nc.scalar
    jt = lambda sh, nm: jp.tile(sh, F32, name=f"{nm}_{p}", tag=nm)

    m = jt([2, 4], "m")
    v.tensor_scalar(m[:], Gpair[:, 16:20], 1.0 / NROWS, None, OP.mult)
    mm = jt([2, 16], "mm")
    ma = m[:].unsqueeze(2).broadcast_to([2, 4, 4])
    mb = m[:].unsqueeze(1).broadcast_to([2, 4, 4])
    v.tensor_tensor(mm[:].rearrange("p (a b) -> p a b", a=4), ma, mb, OP.mult)
    Ac = jt([2, 16], "Ac")
    v.scalar_tensor_tensor(Ac[:], mm[:], -NROWS, Gpair[:, 0:16],
                           OP.mult, OP.add)
    vd = Ac[:, 0:16:5]  # Cov diagonal [2,4]
    sq0 = jt([2, 4], "sq0")
    act.activation(sq0[:], vd, AF.Sqrt)
    sqc = jt([2, 4], "sqc")
    v.tensor_scalar(sqc[:], sq0[:], 1e-30, None, OP.max)
    rv0 = jt([2, 4], "rv0")
    v.reciprocal(rv0[:], sqc[:])
    ud = jt([2, 4], "ud")
    v.tensor_tensor(ud[:], vd, rv0[:], OP.mult)
    s2d = jt([2, 4], "s2d")
    v.tensor_tensor(s2d[:], sqc[:], ud[:], OP.add)  # 2*sqrt(d), refined
    rv = jt([2, 4], "rv")
    v.reciprocal(rv[:], s2d[:])
    mk = jt([2, 4], "mk")
    v.tensor_scalar(mk[:], vd, 0.0, None, OP.is_gt)
    rinv = jt([2, 4], "rinv")
    v.tensor_tensor(rinv[:], rv[:], mk[:], OP.mult)  # 1/(2 sqrt(d)), masked
    rr = jt([2, 16], "rr")
    ra = rinv[:].unsqueeze(2).broadcast_to([2, 4, 4])
    rb = rinv[:].unsqueeze(1).broadcast_to([2, 4, 4])
    v.tensor_tensor(rr[:].rearrange("p (a b) -> p a b", a=4), ra, rb, OP.mult)
    A = jt([2, 16], "A")
    v.tensor_tensor(A[:], Ac[:], rr[:], OP.mult)  # Ghat/4, diag 1/4

    # center + Frobenius shift: B0 = A + (1.1*||A - I/4||_F - 1/4) I
    sqA = jt([2, 16], "sqA")
    v.tensor_tensor(sqA[:], A[:], A[:], OP.mult)
    t = jt([2, 1], "t")
    v.tensor_reduce(t[:], sqA[:], AX.X, OP.add)
    t2 = jt([2, 1], "t2")
    v.tensor_scalar(t2[:], t[:], 1.0, -0.25, OP.mult, OP.add)
    t2c = jt([2, 1], "t2c")
    v.tensor_scalar(t2c[:], t2[:], 1e-30, None, OP.max)
    sf = jt([2, 1], "sf")
    act.activation(sf[:], t2c[:], AF.Sqrt)
    sh = jt([2, 1], "sh")
    v.tensor_scalar(sh[:], sf[:], 1.10, -0.25, OP.mult, OP.add)
    B0 = jt([2, 16], "B0")
    v.tensor_copy(B0[:], A[:])
    v.tensor_scalar(B0[:, 0:16:5], A[:, 0:16:5], sh[:], None, OP.add)
    dmx0 = jt([2, 1], "dmx0")
    v.tensor_reduce(dmx0[:], B0[:, 0:16:5], AX.X, OP.max)
    rp0 = jt([2, 1], "rp0")
    v.reciprocal(rp0[:], dmx0[:])
    Bc = jt([2, 16], "Bc0")
    v.tensor_scalar(Bc[:], B0[:], rp0[:], None, OP.mult)

    # 10 squarings: B <- B @ B (symmetric), renorm by diag max at k=3,7
    Ball = Bc
    for k in range(NSQ):
        colv = (
            Ball[:].rearrange("p (r j) -> p j r", j=4)
            .unsqueeze(3).broadcast_to([2, 4, 4, 4])
        )
        rowv = (
            Ball[:].rearrange("p (j c) -> p j c", j=4)
            .unsqueeze(2).broadcast_to([2, 4, 4, 4])
        )
        P = jp.tile([2, 64], F32, name=f"P{k}_{p}", tag="Psq")
        v.tensor_tensor(P[:].rearrange("p (j r c) -> p j r c", j=4, r=4),
                        colv, rowv, OP.mult)
        Bn = jp.tile([2, 16], F32, name=f"B{k}_{p}", tag="Bsq")
        v.tensor_reduce(Bn[:], P[:].rearrange("p (j rc) -> p rc j", j=4),
                        AX.X, OP.add)
        if k in (3, 7):
            dmx = jp.tile([2, 1], F32, name=f"dmx{k}_{p}", tag="dmx")
            v.tensor_reduce(dmx[:], Bn[:, 0:16:5], AX.X, OP.max)
            rp = jp.tile([2, 1], F32, name=f"rp{k}_{p}", tag="rp")
            v.reciprocal(rp[:], dmx[:])
            Bm = jp.tile([2, 16], F32, name=f"Bm{k}_{p}", tag="Bsq")
            v.tensor_scalar(Bm[:], Bn[:], rp[:], None, OP.mult)
            Ball = Bm
        else:
            Ball = Bn

    # column sums = v * sign(sum(v)) * scale; Newton-normalize
    u4 = jt([2, 4], "u4")
    v.tensor_reduce(u4[:], Ball[:].rearrange("p (r c) -> p r c", r=4),
                    AX.X, OP.add)
    vsq = jt([2, 4], "vsq")
    v.tensor_tensor(vsq[:], u4[:], u4[:], OP.mult)
    n2 = jt([2, 1], "n2")
    v.tensor_reduce(n2[:], vsq[:], AX.X, OP.add)
    s0 = jt([2, 1], "s0")
    act.activation(s0[:], n2[:], AF.Sqrt)
    s0c = jt([2, 1], "s0c")
    v.tensor_scalar(s0c[:], s0[:], 1e-30, None, OP.max)
    r0 = jt([2, 1], "r0")
    v.reciprocal(r0[:], s0c[:])
    un = jt([2, 1], "un")
    v.tensor_tensor(un[:], n2[:], r0[:], OP.mult)
    s2n = jt([2, 1], "s2n")
    v.tensor_tensor(s2n[:], s0c[:], un[:], OP.add)
    rn = jt([2, 1], "rn")
    v.reciprocal(rn[:], s2n[:])  # 1/(2||u||)
    vw = jt([2, 4], "vw")
    v.tensor_scalar(vw[:], u4[:], rn[:], None, OP.mult)  # v/2
    w4 = jt([2, 4], "w4")
    v.scalar_tensor_tensor(w4[:], vw[:], 4.0 * SQRTN, rinv[:],
                           OP.mult, OP.mult)  # v*sqrt(N)/sqrt(d)
    wm = jt([2, 4], "wm")
    v.tensor_tensor(wm[:], w4[:], m[:], OP.mult)
    bs = jt([2, 1], "bs")
    v.tensor_reduce(bs[:], wm[:], AX.X, OP.add)
    wb5 = jt([2, 5], "wb5")
    v.tensor_copy(wb5[:, 0:4], w4[:])
    v.tensor_scalar(wb5[:, 4:5], bs[:], -1.0, None, OP.mult)

    if dbg is not None:
        nc.sync.dma_start(dbg[2 * p : 2 * p + 2, 0:20], Gpair[:])
        nc.sync.dma_start(dbg[2 * p : 2 * p + 2, 20:36], A[:])
        nc.sync.dma_start(dbg[2 * p : 2 * p + 2, 36:52], Ball[:])
        nc.sync.dma_start(dbg[2 * p : 2 * p + 2, 52:56], u4[:])
        nc.sync.dma_start(dbg[2 * p : 2 * p + 2, 56:61], wb5[:])

    # broadcast to all 128 partitions
    wrow = jt([1, 10], "wrow")
    nc.sync.dma_start(wrow[:], wb5[:])
    wbc = jt([128, 10], "wbc")
    nc.sync.dma_start(wbc[:], wrow[:].unsqueeze(1).broadcast_to([1, 128, 10]))
    return wbc


def _project(tc, pools, X5, wbc, y, q, i):
    nc = tc.nc
    v = nc.vector
    act = nc.scalar
    gps = nc.gpsimd
    ppool, rpool = pools
    wv = lambda a: wbc[:, 5 * q + a : 5 * q + a + 1]
    bias = wbc[:, 5 * q + 4 : 5 * q + 5]

    res = rpool.tile([128, 2048], F32, name=f"res{i}", tag="res")
    x3 = X5[:].rearrange("p (f a) -> p f a", a=4)

    # DVE path: groups [0, G1) in fp16: bcast-mult + group-4 reduce + bias
    whf = rpool.tile([128, 4], F16, name=f"whf{i}", tag="whf")
    v.tensor_copy(whf[:], wbc[:, 5 * q : 5 * q + 4])
    prod = ppool.tile([128, G1 * 4], F16, name=f"prod{i}", tag="prod")
    pv3 = prod[:].rearrange("p (f a) -> p f a", a=4)
    whb = whf[:].unsqueeze(1).broadcast_to([128, G1, 4])
    v.tensor_tensor(pv3, x3[:, 0:G1], whb, OP.mult)
    red = ppool.tile([128, G1], F32, name=f"red{i}", tag="red")
    v.tensor_reduce(red[:], pv3, AX.X, OP.add)
    v.tensor_scalar(res[:, 0:G1], red[:], bias, None, OP.add)

    # ACT path: groups [G1, 2048): strided scaled-identity per class + adds
    ms = []
    for a in range(4):
        mt = ppool.tile([128, G2], F32, name=f"m{a}_{i}", tag=f"pm{a}")
        act.activation(mt[:], x3[:, G1:2048, a], AF.Identity,
                       bias=bias if a == 0 else 0.0, scale=wv(a))
        ms.append(mt)
    a01 = ppool.tile([128, G2], F32, name=f"a01_{i}", tag="pa01")
    gps.tensor_tensor(a01[:], ms[0][:], ms[1][:], OP.add)
    a23 = ppool.tile([128, G2], F32, name=f"a23_{i}", tag="pa23")
    gps.tensor_tensor(a23[:], ms[2][:], ms[3][:], OP.add)
    v.tensor_tensor(res[:, G1:2048], a01[:], a23[:], OP.add)

    # output DMA: res free (jl,dy,dx,c4) == y's (jl,c); 8KB HBM runs
    dst = y[i].rearrange("i2 (jh jl) c -> i2 jh (jl c)", jh=2).transpose([1, 0, 2])
    nc.sync.dma_start(dst, res[:])


def _emit(ctx, tc, y, x, maskc, em4c, dbg=None):
    nc = tc.nc
    v = nc.vector

    consts = ctx.enter_context(tc.tile_pool(name="consts", bufs=1))
    xpool = ctx.enter_context(tc.tile_pool(name="xdata", bufs=1))
    gpool = ctx.enter_context(tc.tile_pool(name="gram", bufs=2, space="PSUM"))
    p2pool = ctx.enter_context(tc.tile_pool(name="ps2", bufs=2, space="PSUM"))
    spool = ctx.enter_context(tc.tile_pool(name="small", bufs=2))
    jpool = ctx.enter_context(tc.tile_pool(name="jac", bufs=2))
    ppool = ctx.enter_context(tc.tile_pool(name="proj", bufs=2))
    rpool = ctx.enter_context(tc.tile_pool(name="res", bufs=2))

    mask = consts.tile([128, 128], F32)
    nc.sync.dma_start(mask[:], maskc[:])
    em4 = consts.tile([128, 4], F32)
    nc.sync.dma_start(em4[:], em4c[:])
    ones1 = consts.tile([128, 1], F16)
    nc.gpsimd.memset(ones1[:], 1.0)

    # all loads issued up-front (SWDGE queues drain while PE computes)
    X5 = []
    for i in range(IMGS):
        xi = xpool.tile([128, FREE], F16, name=f"x5img{i}", tag=f"x5_{i}")
        X5.append(xi)
        _load_image(nc, x, xi, i)

    for p in range(2):  # image pairs (0,1) and (2,3)
        Gpair = jpool.tile([2, 20], F32, name=f"gpair{p}", tag="gpair")
        for q in range(2):
            i = 2 * p + q
            gp = _gram(nc, gpool, X5[i][:], ones1, i)
            _fold(ctx, tc, (spool, p2pool), gp, mask, em4, Gpair, q, i)
        wbc = _chain(tc, jpool, Gpair, p, dbg)
        for q in range(2):
            i = 2 * p + q
            _project(tc, (ppool, rpool), X5[i], wbc, y, q, i)


_CACHE = {}


def _build(dbg_mode=False):
    key = "nc_dbg" if dbg_mode else "nc"
    if key in _CACHE:
        return _CACHE[key]
    nc = bacc.Bacc("TRN2", target_bir_lowering=False, debug=False)
    x = nc.dram_tensor("x", [IMGS, H, W, C], F32, kind="ExternalInput").ap()
    maskc = nc.dram_tensor("maskc", [128, 128], F32, kind="ExternalInput").ap()
    em4c = nc.dram_tensor("em4c", [128, 4], F32, kind="ExternalInput").ap()
    y = nc.dram_tensor("y", [IMGS, H // 2, W // 2, C], F32,
                       kind="ExternalOutput").ap()
    dbg = (
        nc.dram_tensor("dbg", [4, 61], F32, kind="ExternalOutput").ap()
        if dbg_mode
        else None
    )
    with tile.TileContext(nc) as tc, ExitStack() as ctx:
        _emit(ctx, tc, y, x, maskc, em4c, dbg)
    nc.compile()
    _CACHE[key] = nc
    return nc


def _consts():
    if "mask" not in _CACHE:
        j = np.arange(128)
        blk = (j[:, None] // 4) == (j[None, :] // 4)
        _CACHE["mask"] = blk.astype(np.float32)
        em = np.zeros((128, 4), dtype=np.float32)
        em[j, j % 4] = 1.0
        _CACHE["em4"] = em
    return _CACHE["mask"], _CACHE["em4"]


def kernel(inputs: np.ndarray, _trace: bool = False, _dbg: bool = False):
    x = np.ascontiguousarray(np.asarray(inputs, dtype=np.float32))
    assert x.shape == (B, H, W, C), x.shape
    nc = _build(_dbg)
    mask, em4 = _consts()
    in_maps = [
        {"x": x[i * IMGS : (i + 1) * IMGS], "maskc": mask, "em4c": em4}
        for i in range(NCORES)
    ]
    res = run_bass_kernel_spmd(
        nc, in_maps, core_ids=list(range(NCORES)), trace=_trace
    )
    out = np.concatenate([res.results[i]["y"] for i in range(NCORES)], axis=0)
    if _trace:
        _CACHE["last_exec_time_ns"] = res.exec_time_ns
        _CACHE["last_results"] = res
    if _dbg:
        _CACHE["last_dbg"] = [res.results[i].get("dbg") for i in range(NCORES)]
    return out


# revision 14
# speedup vs baseline: 1.4270x; 1.4270x over previous
"""BPCA pooling kernel for Trainium2 (Bass/Tile), 8-core data-parallel.

Per core: 4 images [128,128,64] f32.
  1. SWDGE cast-DMA each image HBM->SBUF fp32->fp16 into layout
     [128 part = (hh, jh), free = (dy, jl, dx, c4, a)] so that 4-channel
     classes (a = c%4) are innermost and HBM reads are 16KB-contiguous.
  2. Per-image 4x4 Gram + class sums on PE in fp16 (fp32 PSUM): 64 chunk
     self-matmuls [128,128] accumulate the 32 diagonal 4x4 blocks; an
     interleaved N=1 ones-matmul accumulates per-column sums.
  3. Fold on PE: mask off-diagonal blocks (DVE), em4^T-matmul collapses
     32 diag blocks + the sums column; small DVE reduce; tiny SBUF->SBUF
     gather DMA into image-major [2, 20] pair tiles.
  4. Eigen chain per image-pair on DVE: standardize Gram -> Ghat/4,
     center + Frobenius-shift, 10 power-squarings (B <- B@B via
     broadcast-mult + strided reduce), column-sum = sign-fixed top
     eigenvector, Newton-normalized; w = v/sigma, b = -w.m.
  5. Projection: out = sum_a w_a x_a + b. Front 1280 groups on DVE in
     fp16 (bcast-mult + group-4 reduce + bias), back 768 groups on ACT
     (strided scaled-identity per class) + GPSIMD/DVE adds.
  6. Output DMA [128, 2048] f32 with 8KB-contiguous HBM writes.
"""

import sys
from contextlib import ExitStack

import numpy as np

for _p in ("/opt/trn_rl_repo",):
    if _p not in sys.path:
        sys.path.insert(0, _p)

import concourse.bass as bass
import concourse.bacc as bacc
import concourse.tile as tile
from concourse import mybir
from concourse.bass_utils import run_bass_kernel_spmd

AF = mybir.ActivationFunctionType
OP = mybir.AluOpType
AX = mybir.AxisListType
F32 = mybir.dt.float32
F16 = mybir.dt.float16

B, H, W, C = 32, 128, 128, 64
NCORES = 8
IMGS = B // NCORES  # 4 images per core
NROWS = float(H * W * C // 4)  # 262144 rows per image
SQRTN = float(np.sqrt(NROWS))
FREE = H * W * C // 128  # 8192 elems per partition per image
NCHUNK = FREE // 128  # 64 gram chunks per image
NSQ = 10  # power-squarings on the centered+shifted Ghat
G1 = 1280  # groups on the DVE projection path (of 2048)
G2 = 2048 - G1  # groups on the ACT+GPS projection path


def _load_image(nc, x, X5, i):
    # HBM: x[i] [h=128, w=128, c=64] fp32, h=(hh,dy), w=(jh,jl,dx), c=(c4,a)
    # SBUF: X5 [128=(jh,hh), 8192=(jl,dy,dx,c4,a)] fp16
    # 4 cast-DMAs (dy x jh), each 16KB-contiguous HBM reads per partition.
    for dy in range(2):
        for jh in range(2):
            src = x[i][dy::2, jh * 64 : (jh + 1) * 64, :].rearrange(
                "hh (jl dx) c -> hh jl (dx c)", dx=2
            )  # [64, 32, 128]
            dst = (
                X5[jh * 64 : (jh + 1) * 64]
                .rearrange("p (jl dy r) -> p jl dy r", jl=32, dy=2)[:, :, dy, :]
            )  # [64, 32, 128]
            nc.gpsimd.dma_start(dst, src)


def _gram(nc, gpool, X5, ones1, i):
    gp = gpool.tile([128, 132], F32, name=f"gram{i}", tag="gram")
    for k in range(NCHUNK):
        chunk = X5[:, k * 128 : (k + 1) * 128]
        nc.tensor.matmul(gp[:, 0:128], chunk, chunk,
                         start=(k == 0), stop=False)
        nc.tensor.matmul(gp[:, 128:129], chunk, ones1[:],
                         start=False, stop=(k == NCHUNK - 1))
    return gp


def _fold(ctx, tc, pools, gp, mask, em4, Gpair, q, i):
    nc = tc.nc
    v = nc.vector
    act = nc.scalar
    spool, p2pool = pools
    # PSUM -> SBUF copy on ACT; mask diag blocks on DVE
    gs = spool.tile([128, 129], F32, name=f"gs{i}", tag="gs")
    act.activation(gs[:], gp[:, 0:129], AF.Identity)
    gm = spool.tile([128, 128], F32, name=f"gm{i}", tag="gm")
    v.tensor_tensor(gm[:], gs[:, 0:128], mask[:], OP.mult)
    # fold 32 diag blocks + sums column via em4^T matmul
    ps2 = p2pool.tile([4, 132], F32, name=f"ps2_{i}", tag="ps2")
    nc.tensor.matmul(ps2[:, 0:128], em4[:], gm[:], start=True, stop=False)
    nc.tensor.matmul(ps2[:, 128:129], em4[:], gs[:, 128:129],
                     start=False, stop=True)
    f5 = spool.tile([4, 129], F32, name=f"f5_{i}", tag="f5")
    act.activation(f5[:], ps2[:, 0:129], AF.Identity)
    G4s = spool.tile([4, 4], F32, name=f"g4s{i}", tag="g4s")
    gv = f5[:, 0:128].rearrange("p (j b) -> p j b", b=4).transpose([0, 2, 1])
    v.tensor_reduce(G4s[:], gv, AX.X, OP.add)
    # gather into image-major pair tile: [2, 20] = (G[a,b] at 4a+b, s_a at 16+a)
    # (ACT HWDGE ring: keeps these off the big output-DMA ring)
    act.dma_start(Gpair[q : q + 1, 0:16], G4s[:])
    act.dma_start(Gpair[q : q + 1, 16:20], f5[:, 128:129])


def _chain(tc, jp, Gpair, p, dbg=None):
    """Pair eigen chain on [2,*] tiles -> wbc [128, 10] f32 (w0..w3, b)."""
    nc = tc.nc
    v = nc.vector
    act = nc.scalar
    jt = lambda sh, nm: jp.tile(sh, F32, name=f"{nm}_{p}", tag=nm)

    m = jt([2, 4], "m")
    v.tensor_scalar(m[:], Gpair[:, 16:20], 1.0 / NROWS, None, OP.mult)
    mm = jt([2, 16], "mm")
    ma = m[:].unsqueeze(2).broadcast_to([2, 4, 4])
    mb = m[:].unsqueeze(1).broadcast_to([2, 4, 4])
    v.tensor_tensor(mm[:].rearrange("p (a b) -> p a b", a=4), ma, mb, OP.mult)
    Ac = jt([2, 16], "Ac")
    v.scalar_tensor_tensor(Ac[:], mm[:], -NROWS, Gpair[:, 0:16],
                           OP.mult, OP.add)
    vd = Ac[:, 0:16:5]  # Cov diagonal [2,4]
    sq0 = jt([2, 4], "sq0")
    act.activation(sq0[:], vd, AF.Sqrt)
    sqc = jt([2, 4], "sqc")
    v.tensor_scalar(sqc[:], sq0[:], 1e-30, None, OP.max)
    rv0 = jt([2, 4], "rv0")
    v.reciprocal(rv0[:], sqc[:])
    ud = jt([2, 4], "ud")
    v.tensor_tensor(ud[:], vd, rv0[:], OP.mult)
    s2d = jt([2, 4], "s2d")
    v.tensor_tensor(s2d[:], sqc[:], ud[:], OP.add)  # 2*sqrt(d), refined
    rv = jt([2, 4], "rv")
    v.reciprocal(rv[:], s2d[:])
    mk = jt([2, 4], "mk")
    v.tensor_scalar(mk[:], vd, 0.0, None, OP.is_gt)
    rinv = jt([2, 4], "rinv")
    v.tensor_tensor(rinv[:], rv[:], mk[:], OP.mult)  # 1/(2 sqrt(d)), masked
    rr = jt([2, 16], "rr")
    ra = rinv[:].unsqueeze(2).broadcast_to([2, 4, 4])
    rb = rinv[:].unsqueeze(1).broadcast_to([2, 4, 4])
    v.tensor_tensor(rr[:].rearrange("p (a b) -> p a b", a=4), ra, rb, OP.mult)
    A = jt([2, 16], "A")
    v.tensor_tensor(A[:], Ac[:], rr[:], OP.mult)  # Ghat/4, diag 1/4

    # center + Frobenius shift: B0 = A + (1.1*||A - I/4||_F - 1/4) I
    sqA = jt([2, 16], "sqA")
    v.tensor_tensor(sqA[:], A[:], A[:], OP.mult)
    t = jt([2, 1], "t")
    v.tensor_reduce(t[:], sqA[:], AX.X, OP.add)
    t2 = jt([2, 1], "t2")
    v.tensor_scalar(t2[:], t[:], 1.0, -0.25, OP.mult, OP.add)
    t2c = jt([2, 1], "t2c")
    v.tensor_scalar(t2c[:], t2[:], 1e-30, None, OP.max)
    sf = jt([2, 1], "sf")
    act.activation(sf[:], t2c[:], AF.Sqrt)
    sh = jt([2, 1], "sh")
    v.tensor_scalar(sh[:], sf[:], 1.10, -0.25, OP.mult, OP.add)
    B0 = jt([2, 16], "B0")
    v.tensor_copy(B0[:], A[:])
    v.tensor_scalar(B0[:, 0:16:5], A[:, 0:16:5], sh[:], None, OP.add)
    dmx0 = jt([2, 1], "dmx0")
    v.tensor_reduce(dmx0[:], B0[:, 0:16:5], AX.X, OP.max)
    rp0 = jt([2, 1], "rp0")
    v.reciprocal(rp0[:], dmx0[:])
    Bc = jt([2, 16], "Bc0")
    v.tensor_scalar(Bc[:], B0[:], rp0[:], None, OP.mult)

    # 10 squarings: B <- B @ B (symmetric), renorm by diag max at k=3,7
    Ball = Bc
    for k in range(NSQ):
        colv = (
            Ball[:].rearrange("p (r j) -> p j r", j=4)
            .unsqueeze(3).broadcast_to([2, 4, 4, 4])
        )
        rowv = (
            Ball[:].rearrange("p (j c) -> p j c", j=4)
            .unsqueeze(2).broadcast_to([2, 4, 4, 4])
        )
        P = jp.tile([2, 64], F32, name=f"P{k}_{p}", tag="Psq")
        v.tensor_tensor(P[:].rearrange("p (j r c) -> p j r c", j=4, r=4),
                        colv, rowv, OP.mult)
        Bn = jp.tile([2, 16], F32, name=f"B{k}_{p}", tag="Bsq")
        v.tensor_reduce(Bn[:], P[:].rearrange("p (j rc) -> p rc j", j=4),
                        AX.X, OP.add)
        if k in (3, 7):
            dmx = jp.tile([2, 1], F32, name=f"dmx{k}_{p}", tag="dmx")
            v.tensor_reduce(dmx[:], Bn[:, 0:16:5], AX.X, OP.max)
            rp = jp.tile([2, 1], F32, name=f"rp{k}_{p}", tag="rp")
            v.reciprocal(rp[:], dmx[:])
            Bm = jp.tile([2, 16], F32, name=f"Bm{k}_{p}", tag="Bsq")
            v.tensor_scalar(Bm[:], Bn[:], rp[:], None, OP.mult)
            Ball = Bm
        else:
            Ball = Bn

    # column sums = v * sign(sum(v)) * scale; Newton-normalize
    u4 = jt([2, 4], "u4")
    v.tensor_reduce(u4[:], Ball[:].rearrange("p (r c) -> p r c", r=4),
                    AX.X, OP.add)
    vsq = jt([2, 4], "vsq")
    v.tensor_tensor(vsq[:], u4[:], u4[:], OP.mult)
    n2 = jt([2, 1], "n2")
    v.tensor_reduce(n2[:], vsq[:], AX.X, OP.add)
    s0 = jt([2, 1], "s0")
    act.activation(s0[:], n2[:], AF.Sqrt)
    s0c = jt([2, 1], "s0c")
    v.tensor_scalar(s0c[:], s0[:], 1e-30, None, OP.max)
    r0 = jt([2, 1], "r0")
    v.reciprocal(r0[:], s0c[:])
    un = jt([2, 1], "un")
    v.tensor_tensor(un[:], n2[:], r0[:], OP.mult)
    s2n = jt([2, 1], "s2n")
    v.tensor_tensor(s2n[:], s0c[:], un[:], OP.add)
    rn = jt([2, 1], "rn")
    v.reciprocal(rn[:], s2n[:])  # 1/(2||u||)
    vw = jt([2, 4], "vw")
    v.tensor_scalar(vw[:], u4[:], rn[:], None, OP.mult)  # v/2
    w4 = jt([2, 4], "w4")
    v.scalar_tensor_tensor(w4[:], vw[:], 4.0 * SQRTN, rinv[:],
                           OP.mult, OP.mult)  # v*sqrt(N)/sqrt(d)
    wm = jt([2, 4], "wm")
    v.tensor_tensor(wm[:], w4[:], m[:], OP.mult)
    bs = jt([2, 1], "bs")
    v.tensor_reduce(bs[:], wm[:], AX.X, OP.add)
    wb5 = jt([2, 5], "wb5")
    v.tensor_copy(wb5[:, 0:4], w4[:])
    v.tensor_scalar(wb5[:, 4:5], bs[:], -1.0, None, OP.mult)

    if dbg is not None:
        nc.sync.dma_start(dbg[2 * p : 2 * p + 2, 0:20], Gpair[:])
        nc.sync.dma_start(dbg[2 * p : 2 * p + 2, 20:36], A[:])
        nc.sync.dma_start(dbg[2 * p : 2 * p + 2, 36:52], Ball[:])
        nc.sync.dma_start(dbg[2 * p : 2 * p + 2, 52:56], u4[:])
        nc.sync.dma_start(dbg[2 * p : 2 * p + 2, 56:61], wb5[:])

    # broadcast to all 128 partitions (ACT HWDGE ring)
    wrow = jt([1, 10], "wrow")
    act.dma_start(wrow[:], wb5[:])
    wbc = jt([128, 10], "wbc")
    act.dma_start(wbc[:], wrow[:].unsqueeze(1).broadcast_to([1, 128, 10]))
    return wbc


def _project(tc, pools, X5, wbc, y, q, i):
    nc = tc.nc
    v = nc.vector
    act = nc.scalar
    gps = nc.gpsimd
    ppool, rpool = pools
    wv = lambda a: wbc[:, 5 * q + a : 5 * q + a + 1]
    bias = wbc[:, 5 * q + 4 : 5 * q + 5]

    res = rpool.tile([128, 2048], F32, name=f"res{i}", tag="res")
    x3 = X5[:].rearrange("p (f a) -> p f a", a=4)

    # DVE path: groups [0, G1) in fp16: bcast-mult + group-4 reduce + bias
    whf = rpool.tile([128, 4], F16, name=f"whf{i}", tag="whf")
    v.tensor_copy(whf[:], wbc[:, 5 * q : 5 * q + 4])
    prod = ppool.tile([128, G1 * 4], F16, name=f"prod{i}", tag="prod")
    pv3 = prod[:].rearrange("p (f a) -> p f a", a=4)
    whb = whf[:].unsqueeze(1).broadcast_to([128, G1, 4])
    v.tensor_tensor(pv3, x3[:, 0:G1], whb, OP.mult)
    red = ppool.tile([128, G1], F16, name=f"red{i}", tag="red")
    with nc.allow_low_precision(reason="4-elem fp16 dot; |out|<16, quant 5e-4"):
        v.tensor_reduce(red[:], pv3, AX.X, OP.add)
    v.tensor_scalar(res[:, 0:G1], red[:], bias, None, OP.add)

    # ACT path: groups [G1, 2048): strided scaled-identity per class + adds
    ms = []
    for a in range(4):
        mt = ppool.tile([128, G2], F32, name=f"m{a}_{i}", tag=f"pm{a}")
        act.activation(mt[:], x3[:, G1:2048, a], AF.Identity,
                       bias=bias if a == 0 else 0.0, scale=wv(a))
        ms.append(mt)
    a01 = ppool.tile([128, G2], F32, name=f"a01_{i}", tag="pa01")
    gps.tensor_tensor(a01[:], ms[0][:], ms[1][:], OP.add)
    a23 = ppool.tile([128, G2], F32, name=f"a23_{i}", tag="pa23")
    gps.tensor_tensor(a23[:], ms[2][:], ms[3][:], OP.add)
    v.tensor_tensor(res[:, G1:2048], a01[:], a23[:], OP.add)

    # output DMA: res free (jl,dy,dx,c4) == y's (jl,c); 8KB HBM runs.
    # one DMA per jh half: SBUF partition dim stays leading, DRAM outer
    # dim is 64 so HWDGE splits across all 16 SDMA engines.
    for jh in range(2):
        dst = y[i][:, jh * 32 : (jh + 1) * 32, :].rearrange(
            "i2 jl c -> i2 (jl c)"
        )
        nc.sync.dma_start(dst, res[jh * 64 : (jh + 1) * 64, :])


def _emit(ctx, tc, y, x, maskc, em4c, dbg=None):
    nc = tc.nc
    v = nc.vector

    consts = ctx.enter_context(tc.tile_pool(name="consts", bufs=1))
    xpool = ctx.enter_context(tc.tile_pool(name="xdata", bufs=1))
    gpool = ctx.enter_context(tc.tile_pool(name="gram", bufs=2, space="PSUM"))
    p2pool = ctx.enter_context(tc.tile_pool(name="ps2", bufs=2, space="PSUM"))
    spool = ctx.enter_context(tc.tile_pool(name="small", bufs=2))
    jpool = ctx.enter_context(tc.tile_pool(name="jac", bufs=2))
    ppool = ctx.enter_context(tc.tile_pool(name="proj", bufs=2))
    rpool = ctx.enter_context(tc.tile_pool(name="res", bufs=2))

    mask = consts.tile([128, 128], F32)
    nc.sync.dma_start(mask[:], maskc[:])
    em4 = consts.tile([128, 4], F32)
    nc.sync.dma_start(em4[:], em4c[:])
    ones1 = consts.tile([128, 1], F16)
    nc.gpsimd.memset(ones1[:], 1.0)

    # all loads issued up-front (SWDGE queues drain while PE computes)
    X5 = []
    for i in range(IMGS):
        xi = xpool.tile([128, FREE], F16, name=f"x5img{i}", tag=f"x5_{i}")
        X5.append(xi)
        _load_image(nc, x, xi, i)

    # emission order shapes each engine's FIFO: pair-1 folds are emitted
    # between pair-0's chain and projections so PE/DVE never head-of-line
    # block pair-1 behind pair-0's projection work.
    def gramfold(i, Gpair, q):
        gp = _gram(nc, gpool, X5[i][:], ones1, i)
        _fold(ctx, tc, (spool, p2pool), gp, mask, em4, Gpair, q, i)

    Gpair0 = jpool.tile([2, 20], F32, name="gpair0", tag="gpair")
    Gpair1 = jpool.tile([2, 20], F32, name="gpair1", tag="gpair")
    gramfold(0, Gpair0, 0)
    gramfold(1, Gpair0, 1)
    wbc0 = _chain(tc, jpool, Gpair0, 0, dbg)
    gramfold(2, Gpair1, 0)
    _project(tc, (ppool, rpool), X5[0], wbc0, y, 0, 0)
    gramfold(3, Gpair1, 1)
    wbc1 = _chain(tc, jpool, Gpair1, 1, dbg)
    _project(tc, (ppool, rpool), X5[1], wbc0, y, 1, 1)
    _project(tc, (ppool, rpool), X5[2], wbc1, y, 0, 2)
    _project(tc, (ppool, rpool), X5[3], wbc1, y, 1, 3)


_CACHE = {}


def _build(dbg_mode=False):
    key = "nc_dbg" if dbg_mode else "nc"
    if key in _CACHE:
        return _CACHE[key]
    nc = bacc.Bacc("TRN2", target_bir_lowering=False, debug=False)
    x = nc.dram_tensor("x", [IMGS, H, W, C], F32, kind="ExternalInput").ap()
    maskc = nc.dram_tensor("maskc", [128, 128], F32, kind="ExternalInput").ap()
    em4c = nc.dram_tensor("em4c", [128, 4], F32, kind="ExternalInput").ap()
    y = nc.dram_tensor("y", [IMGS, H // 2, W // 2, C], F32,
                       kind="ExternalOutput").ap()
    dbg = (
        nc.dram_tensor("dbg", [4, 61], F32, kind="ExternalOutput").ap()
        if dbg_mode
        else None
    )
    with tile.TileContext(nc) as tc, ExitStack() as ctx:
        _emit(ctx, tc, y, x, maskc, em4c, dbg)
    nc.compile()
    _CACHE[key] = nc
    return nc


def _consts():
    if "mask" not in _CACHE:
        j = np.arange(128)
        blk = (j[:, None] // 4) == (j[None, :] // 4)
        _CACHE["mask"] = blk.astype(np.float32)
        em = np.zeros((128, 4), dtype=np.float32)
        em[j, j % 4] = 1.0
        _CACHE["em4"] = em
    return _CACHE["mask"], _CACHE["em4"]


def kernel(inputs: np.ndarray, _trace: bool = False, _dbg: bool = False):
    x = np.ascontiguousarray(np.asarray(inputs, dtype=np.float32))
    assert x.shape == (B, H, W, C), x.shape
    nc = _build(_dbg)
    mask, em4 = _consts()
    in_maps = [
        {"x": x[i * IMGS : (i + 1) * IMGS], "maskc": mask, "em4c": em4}
        for i in range(NCORES)
    ]
    res = run_bass_kernel_spmd(
        nc, in_maps, core_ids=list(range(NCORES)), trace=_trace
    )
    out = np.concatenate([res.results[i]["y"] for i in range(NCORES)], axis=0)
    if _trace:
        _CACHE["last_exec_time_ns"] = res.exec_time_ns
        _CACHE["last_results"] = res
    if _dbg:
        _CACHE["last_dbg"] = [res.results[i].get("dbg") for i in range(NCORES)]
    return out
